# revision 1
# baseline (speedup 1.0000x reference)
"""Trainium2 Bass kernel for nn_BornFoward: 200-step leapfrog wave recurrence.

Math (validated against the jax reference in a numpy model):
  - coef = (dt*BGF/dx)^2 is 0.2025 in the interior square [25:167)^2 of the
    192x192 grid and ~4.4e-13 in the outer absorbing ring; rf is EXACTLY zero
    outside the central 96x96 window (pad region has X==1 -> 1-X^2==0).
  - Therefore the recurrence restricted to the 142x142 interior with zero
    Dirichlet boundary and constant coef reproduces the reference to ~1e-9.
  - p_new = 2*p1 - p0 + C*lap4(p1) + rf*d2(P0),  meas = p_new at 32 pixels.

Sharding: 16 independent recurrences (B=2 x NR=8) -> channel r per core,
both batches per core, batched along the matmul free (column) dimension.

Layout per core: state tiles [71 partitions, 2 chunks x 292], where each
chunk holds rows 71k..71k+70 as two field segments [2 guard | 142 | 2 guard].
All matmul rhs operands are contiguous 292-column runs (N=292 >= 256 so
float32r matmuls stream at 1 cycle/row).

Per-core per-step compute:
  PSUM_m = band_x @ p1          (x-stencil + diag + 2I; 2 K-chunks; +32 meas
                                 selection rows augmented onto chunk-0 lhsT)
         + a*I @ p1(cols+-1) + b*I @ p1(cols+-2)    (y-stencil, shifted rhs)
         + I @ G[j]                                  (host-precomputed rf*d2)
  p_new  = PSUM_m - p0          (DVE fused copyback, rotates state tiles)
  meas   = per-field one-hot mask-reduce (STT accum) of the selection rows.
"""
import sys
import os
import numpy as np
from contextlib import ExitStack

sys.path.insert(0, "/opt/trn_rl_repo")

# ---- problem constants (hardcoded; kernel.py must be self-contained) ----
NX = 192
NT = 200
dtime = 0.3
nm, sR = 32, 70
bg = 1.5
LO, HI = 25, 167            # interior rows/cols [LO, HI) -> D = 142
D = HI - LO
CLO, CHI = 48, 144          # central rf-support window (96 wide)
CW = CHI - CLO
COFF = CLO - LO             # 23: central window offset inside domain
C = (dtime * bg / 1.0) ** 2  # 0.2025
K = 71                      # row-chunk size (2 chunks of 71 = 142)
SEG = 2 + D + 2             # 146: per-field segment with 2-col guards
CHW = 2 * SEG               # 292: chunk width (two fields)
NRR = 8
BB = 2
NMEAS = nm

_thetas = 2 * np.pi * np.arange(nm) / nm
_MX = (NX / 2 + sR * np.cos(_thetas)).astype(int)
_MY = (NX / 2 + sR * np.sin(_thetas)).astype(int)

INCLUDE_2I = True           # fold the 2*p1 term into the band matmul

_prog_cache = {}


def _build_band_consts():
    """Host-side constant matrices for the matmuls (numpy float32)."""
    S = np.zeros((D, D), np.float32)
    idx = np.arange(D)
    S[idx, idx] = -60.0 * C / 12.0 + (2.0 if INCLUDE_2I else 0.0)
    S[idx[:-1], idx[:-1] + 1] = 16.0 * C / 12.0
    S[idx[1:], idx[1:] - 1] = 16.0 * C / 12.0
    S[idx[:-2], idx[:-2] + 2] = -C / 12.0
    S[idx[2:], idx[2:] - 2] = -C / 12.0

    BD = {}
    for kc in range(2):
        for mc in range(2):
            blk = S[mc * K:(mc + 1) * K, kc * K:(kc + 1) * K].T.copy()
            if mc == 0:
                aug = np.zeros((K, 96 + NMEAS), np.float32)
                aug[:, :K] = blk
                for i in range(NMEAS):
                    g = _MX[i] - LO
                    if g // K == kc:
                        aug[g % K, 96 + i] = 1.0
                blk = aug
            BD[(kc, mc)] = np.ascontiguousarray(blk)

    IG = np.eye(K, dtype=np.float32)
    SH1 = np.eye(K, dtype=np.float32) * np.float32(16.0 * C / 12.0)
    SH2 = np.eye(K, dtype=np.float32) * np.float32(-C / 12.0)

    # per-field one-hot masks over the 142 data cols
    MASK = np.zeros((NMEAS, D), np.float32)
    for i in range(NMEAS):
        MASK[i, _MY[i] - LO] = 1.0
    return BD, SH1, SH2, IG, MASK


def _build_program(nt=NT, debug=False, reps=1):
    import concourse.bacc as bacc
    import concourse.tile as tile
    import concourse.mybir as mybir

    dt = mybir.dt
    nc = bacc.Bacc("TRN2", target_bir_lowering=False)

    G_d = nc.dram_tensor("G", (NT, BB, CW, CW), dt.float32r, kind="ExternalInput")
    BD_d = {
        (kc, mc): nc.dram_tensor(
            f"BD{kc}{mc}", (K, (96 + NMEAS) if mc == 0 else K), dt.float32r,
            kind="ExternalInput")
        for kc in range(2) for mc in range(2)
    }
    SH1_d = nc.dram_tensor("SH1", (K, K), dt.float32r, kind="ExternalInput")
    SH2_d = nc.dram_tensor("SH2", (K, K), dt.float32r, kind="ExternalInput")
    IG_d = nc.dram_tensor("IG", (K, K), dt.float32r, kind="ExternalInput")
    MASK_d = nc.dram_tensor("MASK", (NMEAS, D), dt.float32, kind="ExternalInput")
    ZERO_d = nc.dram_tensor("ZERO", (K, 300), dt.float32r, kind="ExternalInput")
    OUT_d = nc.dram_tensor("OUT", (BB, NMEAS, NT), dt.float32, kind="ExternalOutput")
    if debug:
        DBGC_d = nc.dram_tensor("DBGC", (2, K, 300), dt.float32, kind="ExternalOutput")
        DBGP_d = nc.dram_tensor("DBGP", (2, K, 300), dt.float32, kind="ExternalOutput")

    GPF = 3  # G stream ring depth
    PAD = 4  # left/right pad so shift offsets stay in-bounds

    with tile.TileContext(nc) as tc, ExitStack() as ctx:
        def sbuf(name, shape, dty):
            return ctx.enter_context(nc.sbuf_tensor(name, shape, dty))

        # per-chunk state tiles: [4 pad | 2 x (2+142+2) | 4 pad] = 300 cols
        PA = [sbuf(f"PA{kc}", [K, 300], dt.float32r) for kc in range(2)]
        PB = [sbuf(f"PB{kc}", [K, 300], dt.float32r) for kc in range(2)]
        # G ring: per chunk, state layout (zero-padded, central cols DMA'd)
        Gr = [[sbuf(f"Gr{i}_{kc}", [K, 300], dt.float32r) for kc in range(2)]
              for i in range(GPF)]
        bd_t = {km: sbuf(f"bd{km[0]}{km[1]}",
                         [K, (96 + NMEAS) if km[1] == 0 else K], dt.float32r)
                for km in BD_d}
        sh1_t = sbuf("sh1", [K, K], dt.float32r)
        sh2_t = sbuf("sh2", [K, K], dt.float32r)
        ig_t = sbuf("ig", [K, K], dt.float32r)
        mask_t = sbuf("mask", [NMEAS, D], dt.float32)
        meas_t = sbuf("meas", [NMEAS, BB * NT], dt.float32)
        scr_t = sbuf("scr", [NMEAS, D], dt.float32)

        ps_pool = ctx.enter_context(tc.tile_pool(name="ps", bufs=2, space="PSUM"))

        for kc in range(2):
            nc.sync.dma_start(PA[kc][:], ZERO_d[:])
            nc.sync.dma_start(PB[kc][:], ZERO_d[:])
        for i in range(GPF):
            for kc in range(2):
                nc.sync.dma_start(Gr[i][kc][:], ZERO_d[:])
        nc.vector.memset(meas_t[:], 0.0)
        for km, d in BD_d.items():
            nc.sync.dma_start(bd_t[km][:], d[:])
        nc.sync.dma_start(sh1_t[:], SH1_d[:])
        nc.sync.dma_start(sh2_t[:], SH2_d[:])
        nc.sync.dma_start(ig_t[:], IG_d[:])
        nc.sync.dma_start(mask_t[:], MASK_d[:])

        def g_dma(j):
            """DMA G[j] (BB, 96, 96) into ring slot j%GPF, chunk-aligned."""
            for kc in range(2):
                gt = Gr[j % GPF][kc]
                plo = COFF if kc == 0 else 0          # partition base
                rlo = 0 if kc == 0 else 48            # central row base
                src = G_d[j, :, rlo:rlo + 48, :].rearrange("f r c -> r f c")
                dst = gt[plo:plo + 48, PAD:PAD + CHW].rearrange(
                    "p (f c) -> p f c", c=SEG)[:, :, 2 + COFF:2 + COFF + CW]
                nc.sync.dma_start(dst, src)

        def run_view(t, off=0):
            """Contiguous [71, 292] matmul-rhs view at col-tap off."""
            return t[:, PAD + off: PAD + off + CHW]

        def data_view(t, cast_f32=False):
            """[71, 2(field), 142] data view (for DVE ops)."""
            v = t[:, PAD:PAD + CHW]
            if cast_f32:
                v = v.bitcast(dt.float32)
            return v.rearrange("p (f c) -> p f c", c=SEG)[:, :, 2:2 + D]

        def central_view(t, cast_f32=False):
            """[71, 2(field), 96] central-cols view of a state chunk tile."""
            v = t[:, PAD:PAD + CHW]
            if cast_f32:
                v = v.bitcast(dt.float32)
            return v.rearrange("p (f c) -> p f c", c=SEG)[
                :, :, 2 + COFF:2 + COFF + CW]

        def meas_extract(pt, j):
            """Extract 32x2 measurements for output step j from selection rows."""
            for f in range(2):
                seg = pt[96:96 + NMEAS, f * SEG + 2: f * SEG + 2 + D]
                nc.vector.scalar_tensor_tensor(
                    out=scr_t[:], in0=seg, scalar=1.0, in1=mask_t[:],
                    op0=mybir.AluOpType.mult, op1=mybir.AluOpType.mult,
                    accum_out=meas_t[:, f * NT + j: f * NT + j + 1],
                )

        cur, prev = PA, PB
        for rep in range(reps):
          if rep > 0:
            # re-zero state so values stay bounded across timing reps
            for kc in range(2):
                nc.sync.dma_start(PA[kc][:], ZERO_d[:])
                nc.sync.dma_start(PB[kc][:], ZERO_d[:])
          for j in range(nt):
              if j == 0:
                  for q in range(min(GPF - 1, nt)):
                      g_dma(q)
              if j + GPF - 1 < nt:
                  g_dma(j + GPF - 1)

              psums = []
              for mc in range(2):
                  mrows = 128 if mc == 0 else K
                  pt = ps_pool.tile([mrows, CHW], dt.float32, tag=f"ps{mc}")
                  pd = pt[0:K, :]
                  full = pt[:] if mc == 0 else pd
                  nc.tensor.matmul(full, bd_t[(0, mc)][:], run_view(cur[0]),
                                   start=True, stop=False)
                  nc.tensor.matmul(pd, sh1_t[:], run_view(cur[mc], -1),
                                   start=False, stop=False)
                  nc.tensor.matmul(pd, sh1_t[:], run_view(cur[mc], 1),
                                   start=False, stop=False)
                  nc.tensor.matmul(pd, sh2_t[:], run_view(cur[mc], -2),
                                   start=False, stop=False)
                  nc.tensor.matmul(pd, sh2_t[:], run_view(cur[mc], 2),
                                   start=False, stop=False)
                  nc.tensor.matmul(pd, ig_t[:], run_view(Gr[j % GPF][mc]),
                                   start=False, stop=False)
                  nc.tensor.matmul(full, bd_t[(1, mc)][:], run_view(cur[1]),
                                   start=False, stop=True)
                  psums.append(pt)

              for mc in range(2):
                  pd = psums[mc][0:K, :].rearrange(
                      "p (f c) -> p f c", c=SEG)[:, :, 2:2 + D]
                  nc.vector.tensor_tensor(
                      out=data_view(prev[mc]), in0=pd,
                      in1=data_view(prev[mc], cast_f32=True),
                      op=mybir.AluOpType.subtract)

              if j > 0:
                  meas_extract(psums[0], j - 1)

              cur, prev = prev, cur

        # final measurement for output step nt-1 on the final state
        pt = ps_pool.tile([128, CHW], dt.float32, tag="ps0")
        nc.tensor.matmul(pt[:], bd_t[(0, 0)][:], run_view(cur[0]),
                         start=True, stop=False)
        nc.tensor.matmul(pt[:], bd_t[(1, 0)][:], run_view(cur[1]),
                         start=False, stop=True)
        meas_extract(pt, nt - 1)

        if debug:
            for kc in range(2):
                nc.sync.dma_start(DBGC_d[kc], cur[kc][:].bitcast(dt.float32))
                nc.sync.dma_start(DBGP_d[kc], prev[kc][:].bitcast(dt.float32))
        nc.sync.dma_start(
            OUT_d[:].rearrange("f i j -> i f j"),
            meas_t[:].rearrange("i (f j) -> i f j", j=NT))

    nc.compile()
    return nc


def kernel(x, P0):
    x = np.asarray(x, dtype=np.float32)
    P0 = np.asarray(P0, dtype=np.float32)
    from concourse.bass_utils import run_bass_kernel_spmd

    if "prog" not in _prog_cache:
        _prog_cache["prog"] = _build_program()
    nc = _prog_cache["prog"]

    BD, SH1, SH2, IG, MASK = _build_band_consts()

    xx = bg / x[:, 0]
    rf = (1.0 - xx * xx).astype(np.float32)           # (B, 96, 96)
    P0c = P0[0, :, :, CLO:CHI, CLO:CHI]               # (NR, NT, 96, 96)
    d2 = np.zeros_like(P0c)
    d2[:, 2:] = P0c[:, 2:] - 2.0 * P0c[:, 1:-1] + P0c[:, :-2]

    consts = {"SH1": SH1, "SH2": SH2, "IG": IG, "MASK": MASK,
              "ZERO": np.zeros((K, 300), np.float32)}
    for km, v in BD.items():
        consts[f"BD{km[0]}{km[1]}"] = v

    in_maps = []
    for r in range(NRR):
        G = (rf[None, :, :, :] * d2[r][:, None, :, :]).astype(np.float32)
        m = dict(consts)
        m["G"] = np.ascontiguousarray(G)
        in_maps.append(m)

    trace = bool(int(os.environ.get("KERNEL_TRACE", "0")))
    res = run_bass_kernel_spmd(nc, in_maps, core_ids=list(range(NRR)),
                               trace=trace)
    _prog_cache["last_result"] = res
    out = np.zeros((BB, NRR, NMEAS, NT), np.float32)
    for r in range(NRR):
        out[:, r] = res.results[r]["OUT"]
    return out



# revision 16
# speedup vs baseline: 1.1283x; 1.1283x over previous
"""Trainium2 Bass kernel for nn_BornFoward: 200-step leapfrog wave recurrence.

Math (validated against the jax reference in a numpy model):
  - coef = (dt*BGF/dx)^2 is 0.2025 in the interior square [25:167)^2 of the
    192x192 grid and ~4.4e-13 in the outer absorbing ring; rf is EXACTLY zero
    outside the central 96x96 window (pad region has X==1 -> 1-X^2==0).
  - Therefore the recurrence restricted to the 142x142 interior with zero
    Dirichlet boundary and constant coef reproduces the reference to ~1e-9.
  - p_new = 2*p1 - p0 + C*lap4(p1) + rf*d2(P0),  meas = p_new at 32 pixels.
  - fp16 state/weights/source keep the end-to-end error at ~9e-3 of the
    output scale (validated in numpy), under the 2e-2 gate.

Sharding: 16 independent recurrences (B=2 x NR=8) -> channel r per core.

The two batch fields are INDEPENDENT recurrences: they are interleaved as
two separate matmul streams (fp16 runs 1 cycle/row at any free size, so the
N>=256 float32r constraint that forced batching them in one 292-wide stream
is gone). Each field's PSUM->DVE-copyback->next-matmul latency (~900ns)
hides under the other field's 12-matmul block.

Per field, per step (parity p = j%2, PSUM bank tensor PS[f][chunk][p]):
  - state tiles [119, 2(chunk), 154] fp16: partitions 0..70 hold rows of the
    chunk ([4 pad | 2 guard | 142 | 2 guard | 4 pad] cols); partitions
    71..118 are a G ANNEX holding the 48 central rows of rf*d2 for this
    step, zero outside the central cols. The band lhsT is extended with 48
    identity rows, so the source term rides the band matmul's contraction
    dim -- G delivery costs zero extra PE/DVE/Act time; it is DMA'd from
    DRAM straight into the annex (1 DMA per field per step, 2-step lookahead
    into the tile that will be cur again).
  - 12 matmuls: band+G+aug (start=True zeroes the bank), 4 shifted-identity
    y-stencil taps, cross band from the other chunk (stop=True), x2 chunks.
    The chunk-0 band matmuls carry 32 augmented selection columns that copy
    the receiver rows of the current state into PSUM rows 96..127.
  - p_new = PSUM - p0: DVE copyback per chunk (rotates state tiles).
  - the Activation engine copies the [32, 146] selection rows into a big
    on-chip measurement buffer (fp16, lossless: they are copies of fp16
    state values); ONE DMA at the end ships all measurements; the host
    picks each receiver's column. No per-step measurement DMA/mask-reduce.
"""
import sys
import os
import numpy as np
from contextlib import ExitStack

sys.path.insert(0, "/opt/trn_rl_repo")

# ---- problem constants (hardcoded; kernel.py must be self-contained) ----
NX = 192
NT = 200
dtime = 0.3
nm, sR = 32, 70
bg = 1.5
LO, HI = 25, 167            # interior rows/cols [LO, HI) -> D = 142
D = HI - LO
CLO, CHI = 48, 144          # central rf-support window (96 wide)
CW = CHI - CLO
COFF = CLO - LO             # 23: central window offset inside domain
C = (dtime * bg / 1.0) ** 2  # 0.2025
K = 71                      # row-chunk size (2 chunks of 71 = 142)
KA = K + 48                 # 119: chunk rows + G annex rows
SEG = 2 + D + 2             # 146: per-chunk col run with 2-col guards
NRR = 8
BB = 2
NMEAS = nm
PAD = 4                     # left/right pad so shift views stay in-bounds
TW = PAD + SEG + PAD        # 154: state tile cols per chunk

_thetas = 2 * np.pi * np.arange(nm) / nm
_MX = (NX / 2 + sR * np.cos(_thetas)).astype(int)
_MY = (NX / 2 + sR * np.sin(_thetas)).astype(int)

_prog_cache = {}


def _build_band_consts():
    """Host-side constant matrices for the matmuls (numpy float16)."""
    S = np.zeros((D, D), np.float32)
    idx = np.arange(D)
    S[idx, idx] = -60.0 * C / 12.0 + 2.0     # 2*p1 folded into the diagonal
    S[idx[:-1], idx[:-1] + 1] = 16.0 * C / 12.0
    S[idx[1:], idx[1:] - 1] = 16.0 * C / 12.0
    S[idx[:-2], idx[:-2] + 2] = -C / 12.0
    S[idx[2:], idx[2:] - 2] = -C / 12.0

    # BD00 [119,128]: band chunk0->chunk0 + aug selection + G-annex identity
    BD00 = np.zeros((KA, 128), np.float16)
    BD00[0:K, 0:K] = S[0:K, 0:K].T
    # BD10 [71,128]: band chunk1->chunk0 (cross) + aug selection
    BD10 = np.zeros((K, 128), np.float16)
    BD10[0:K, 0:K] = S[0:K, K:2 * K].T
    for i in range(NMEAS):
        g = _MX[i] - LO
        if g < K:
            BD00[g, 96 + i] = 1.0
        else:
            BD10[g - K, 96 + i] = 1.0
    for s in range(48):
        BD00[K + s, COFF + s] = 1.0          # G chunk0: out row 23+s
    # BD01 [71,71]: band chunk0->chunk1 (cross)
    BD01 = np.ascontiguousarray(S[K:2 * K, 0:K].T.astype(np.float16))
    # BD11 [119,71]: band chunk1->chunk1 + G-annex identity
    BD11 = np.zeros((KA, K), np.float16)
    BD11[0:K, 0:K] = S[K:2 * K, K:2 * K].T
    for s in range(48):
        BD11[K + s, s] = 1.0                 # G chunk1: out row s (=71+s)

    SH1 = (np.eye(K) * (16.0 * C / 12.0)).astype(np.float16)
    SH2 = (np.eye(K) * (-C / 12.0)).astype(np.float16)
    return {"BD00": BD00, "BD10": BD10, "BD01": BD01, "BD11": BD11,
            "SH1": SH1, "SH2": SH2}


def _build_program(nt=NT, reps=1):
    import concourse.bacc as bacc
    import concourse.tile as tile
    import concourse.mybir as mybir

    dt = mybir.dt
    nc = bacc.Bacc("TRN2", target_bir_lowering=False)

    G_d = nc.dram_tensor("G", (NT, BB, 2, 48, CW), dt.float16,
                         kind="ExternalInput")
    CSHAPES = [("BD00", (KA, 128)), ("BD10", (K, 128)),
               ("BD01", (K, K)), ("BD11", (KA, K)),
               ("SH1", (K, K)), ("SH2", (K, K))]
    CWID = sum(s[1][1] for s in CSHAPES)
    CP_d = nc.dram_tensor("CPACK", (KA, CWID), dt.float16,
                          kind="ExternalInput")
    MEAS_d = nc.dram_tensor("MEAS", (BB, NMEAS, NT, SEG), dt.float16,
                            kind="ExternalOutput")

    with tile.TileContext(nc) as tc, ExitStack() as ctx:
        def sbuf(name, shape, dty):
            return ctx.enter_context(nc.sbuf_tensor(name, shape, dty))

        # state tiles: [chunk rows + G annex, chunk, cols]; T[f][s]
        T = [[sbuf(f"T{f}{s}", [KA, 2, TW], dt.float16) for s in range(2)]
             for f in range(BB)]
        cpack = sbuf("cpack", [KA, CWID], dt.float16)
        ct, _co = {}, 0
        for n, shp in CSHAPES:
            ct[n] = cpack[0:shp[0], _co:_co + shp[1]]
            _co += shp[1]
        msb = [sbuf(f"msb{f}", [NMEAS, NT, SEG], dt.float16)
               for f in range(BB)]

        # 8 one-bank PSUM tensors: PS[field][chunk][parity]
        PS = [[[ctx.enter_context(
                    nc.psum_tensor(f"PS{f}{kc}{p}", [128, 512], dt.float32))
                for p in range(2)] for kc in range(2)] for f in range(BB)]

        for f in range(BB):
            for s in range(2):
                (nc.vector if f == 0 else nc.gpsimd).memset(T[f][s][:], 0.0)
        nc.sync.dma_start(cpack[:], CP_d[:])

        def g_dma(q, f, s):
            """DMA G[q] for field f into tile slot s's annex (both chunks)."""
            nc.sync.dma_start(
                T[f][s][K:KA, 0:2, PAD + 2 + COFF: PAD + 2 + COFF + CW],
                G_d[q, f].rearrange("k p c -> p k c"))

        def rv(t, kc, off=0, annex=False):
            """Matmul-rhs view: [71 or 119, 146] run at col-tap off."""
            return t[0:(KA if annex else K), kc, PAD + off: PAD + off + SEG]

        cur, prev = 0, 1
        for rep in range(reps):
          if rep > 0:
            for f in range(BB):
                for s in range(2):
                    (nc.vector if f == 0 else nc.gpsimd).memset(T[f][s][:], 0.0)
          for j in range(nt):
              p = j % 2
              if j == 0:
                  for f in range(BB):
                      g_dma(0, f, cur)
                      if nt > 1:
                          g_dma(1, f, prev)

              for f in range(BB):
                  tc_, tp_ = T[f][cur], T[f][prev]
                  O0 = PS[f][0][p]
                  O1 = PS[f][1][p]
                  mm = nc.tensor.matmul
                  kw = dict(start=False, stop=False, skip_group_check=True)
                  # chunk0: band+G+aug opens the bank; cross (needs the other
                  # chunk's copyback of last step) closes it as late as ok.
                  mm(O0[0:128, 0:SEG], ct["BD00"], rv(tc_, 0, annex=True),
                     start=True, stop=False, skip_group_check=True)
                  mm(O0[0:K, 0:SEG], ct["SH1"], rv(tc_, 0, -1), **kw)
                  mm(O0[0:K, 0:SEG], ct["SH1"], rv(tc_, 0, 1), **kw)
                  mm(O0[0:K, 0:SEG], ct["SH2"], rv(tc_, 0, -2), **kw)
                  mm(O0[0:K, 0:SEG], ct["SH2"], rv(tc_, 0, 2), **kw)
                  mm(O1[0:K, 0:SEG], ct["BD01"], rv(tc_, 0),
                     start=True, stop=False, skip_group_check=True)
                  mm(O0[0:128, 0:SEG], ct["BD10"], rv(tc_, 1),
                     start=False, stop=True, skip_group_check=True)
                  mm(O1[0:K, 0:SEG], ct["SH1"], rv(tc_, 1, -1), **kw)
                  mm(O1[0:K, 0:SEG], ct["SH1"], rv(tc_, 1, 1), **kw)
                  mm(O1[0:K, 0:SEG], ct["SH2"], rv(tc_, 1, -2), **kw)
                  mm(O1[0:K, 0:SEG], ct["SH2"], rv(tc_, 1, 2), **kw)
                  mm(O1[0:K, 0:SEG], ct["BD11"], rv(tc_, 1, annex=True),
                     start=False, stop=True, skip_group_check=True)

                  for kc in range(2):
                      dv = tp_[0:K, kc, PAD + 2: PAD + 2 + D]
                      nc.vector.tensor_tensor(
                          out=dv, in0=PS[f][kc][p][0:K, 2:2 + D], in1=dv,
                          op=mybir.AluOpType.subtract)

                  if rep == reps - 1 and j > 0:
                      nc.scalar.copy(msb[f][:, j - 1, :],
                                     PS[f][0][p][96:96 + NMEAS, 0:SEG])

              if rep == reps - 1 and j > 16 and j % 16 == 1:
                  b = j // 16 - 1
                  for f in range(BB):
                      nc.gpsimd.dma_start(MEAS_d[f][:, 16 * b:16 * b + 16, :],
                                          msb[f][:, 16 * b:16 * b + 16, :])

              if j + 2 < nt:
                  for f in range(BB):
                      g_dma(j + 2, f, cur)

              cur, prev = prev, cur

        # post-loop: one more aug pair per field for p^(nt) -> slot nt-1
        for f in range(BB):
            O0 = PS[f][0][nt % 2]
            nc.tensor.matmul(O0[0:128, 0:SEG], ct["BD00"],
                             rv(T[f][cur], 0, annex=True),
                             start=True, stop=False, skip_group_check=True)
            nc.tensor.matmul(O0[0:128, 0:SEG], ct["BD10"],
                             rv(T[f][cur], 1),
                             start=False, stop=True, skip_group_check=True)
            nc.scalar.copy(msb[f][:, nt - 1, :],
                           O0[96:96 + NMEAS, 0:SEG])
        done = max(0, 16 * ((nt - 2) // 16 - (1 if (nt - 2) % 16 == 0 else 0)))
        done = 16 * max(0, (nt - 17) // 16)
        for f in range(BB):
            nc.sync.dma_start(MEAS_d[f][:, done:nt, :], msb[f][:, done:nt, :])

    nc.compile()
    return nc


def kernel(x, P0):
    x = np.asarray(x, dtype=np.float32)
    P0 = np.asarray(P0, dtype=np.float32)
    from concourse.bass_utils import run_bass_kernel_spmd

    if "prog" not in _prog_cache:
        _prog_cache["prog"] = _build_program()
    nc = _prog_cache["prog"]

    cb = _build_band_consts()
    order = ["BD00", "BD10", "BD01", "BD11", "SH1", "SH2"]
    wid = sum(cb[n].shape[1] for n in order)
    cp = np.zeros((KA, wid), np.float16)
    co = 0
    for n in order:
        a = cb[n]
        cp[0:a.shape[0], co:co + a.shape[1]] = a
        co += a.shape[1]
    consts = {"CPACK": cp}

    xx = bg / x[:, 0]
    rf = (1.0 - xx * xx).astype(np.float32)           # (B, 96, 96)
    P0c = P0[0, :, :, CLO:CHI, CLO:CHI]               # (NR, NT, 96, 96)
    d2 = np.zeros_like(P0c)
    d2[:, 2:] = P0c[:, 2:] - 2.0 * P0c[:, 1:-1] + P0c[:, :-2]

    in_maps = []
    for r in range(NRR):
        Gc = (rf[None, :, :, :] * d2[r][:, None, :, :]).astype(np.float16)
        G = np.zeros((NT, BB, 2, 48, CW), np.float16)
        G[:, :, 0] = Gc[:, :, 0:48, :]                # central rows 0..47
        G[:, :, 1] = Gc[:, :, 48:96, :]               # central rows 48..95
        m = dict(consts)
        m["G"] = G
        in_maps.append(m)

    trace = bool(int(os.environ.get("KERNEL_TRACE", "0")))
    res = run_bass_kernel_spmd(nc, in_maps, core_ids=list(range(NRR)),
                               trace=trace)
    _prog_cache["last_result"] = res

    ry = _MY - LO
    ii = np.arange(NMEAS)
    out = np.zeros((BB, NRR, NMEAS, NT), np.float32)
    for r in range(NRR):
        Ms = res.results[r]["MEAS"]                   # (BB, 32, NT, SEG) f16
        for f in range(BB):
            out[f, r] = Ms[f][ii, :, 2 + ry].astype(np.float32)
    return out


# revision 29
# speedup vs baseline: 1.1298x; 1.0014x over previous
"""Trainium2 Bass kernel for nn_BornFoward: 200-step leapfrog wave recurrence.

Math (validated against the jax reference in a numpy model):
  - coef = (dt*BGF/dx)^2 is 0.2025 in the interior square [25:167)^2 of the
    192x192 grid and ~4.4e-13 in the outer absorbing ring; rf is EXACTLY zero
    outside the central 96x96 window (pad region has X==1 -> 1-X^2==0).
  - Therefore the recurrence restricted to the 142x142 interior with zero
    Dirichlet boundary and constant coef reproduces the reference to ~1e-9.
  - p_new = 2*p1 - p0 + C*lap4(p1) + rf*d2(P0),  meas = p_new at 32 pixels.
  - fp16 state/weights/source keep the end-to-end error at ~9e-3 of the
    output scale (validated in numpy), under the 2e-2 gate.

Sharding: 16 independent recurrences (B=2 x NR=8) -> channel r per core.

The two batch fields are INDEPENDENT recurrences: they are interleaved as
two separate matmul streams (fp16 runs 1 cycle/row at any free size, so the
N>=256 float32r constraint that forced batching them in one 292-wide stream
is gone). Each field's PSUM->DVE-copyback->next-matmul latency (~900ns)
hides under the other field's 12-matmul block.

Per field, per step (parity p = j%2, PSUM bank tensor PS[f][chunk][p]):
  - state tiles [119, 2(chunk), 154] fp16: partitions 0..70 hold rows of the
    chunk ([4 pad | 2 guard | 142 | 2 guard | 4 pad] cols); partitions
    71..118 are a G ANNEX holding the 48 central rows of rf*d2 for this
    step, zero outside the central cols. The band lhsT is extended with 48
    identity rows, so the source term rides the band matmul's contraction
    dim -- G delivery costs zero extra PE/DVE/Act time; it is DMA'd from
    DRAM straight into the annex (1 DMA per field per step, 2-step lookahead
    into the tile that will be cur again).
  - 12 matmuls: band+G+aug (start=True zeroes the bank), 4 shifted-identity
    y-stencil taps, cross band from the other chunk (stop=True), x2 chunks.
    The chunk-0 band matmuls carry 32 augmented selection columns that copy
    the receiver rows of the current state into PSUM rows 96..127.
  - p_new = PSUM - p0: DVE copyback per chunk (rotates state tiles).
  - the Activation engine copies the [32, 146] selection rows into a big
    on-chip measurement buffer (fp16, lossless: they are copies of fp16
    state values); ONE DMA at the end ships all measurements; the host
    picks each receiver's column. No per-step measurement DMA/mask-reduce.
"""
import sys
import os
import numpy as np
from contextlib import ExitStack

sys.path.insert(0, "/opt/trn_rl_repo")

# ---- problem constants (hardcoded; kernel.py must be self-contained) ----
NX = 192
NT = 200
dtime = 0.3
nm, sR = 32, 70
bg = 1.5
LO, HI = 25, 167            # interior rows/cols [LO, HI) -> D = 142
D = HI - LO
CLO, CHI = 48, 144          # central rf-support window (96 wide)
CW = CHI - CLO
COFF = CLO - LO             # 23: central window offset inside domain
C = (dtime * bg / 1.0) ** 2  # 0.2025
K = 71                      # row-chunk size (2 chunks of 71 = 142)
KA = K + 48                 # 119: chunk rows + G annex rows
SEG = 2 + D + 2             # 146: per-chunk col run with 2-col guards
NRR = 8
BB = 2
NMEAS = nm
PAD = 4                     # left/right pad so shift views stay in-bounds
TW = PAD + SEG + PAD        # 154: state tile cols per chunk

_thetas = 2 * np.pi * np.arange(nm) / nm
_MX = (NX / 2 + sR * np.cos(_thetas)).astype(int)
_MY = (NX / 2 + sR * np.sin(_thetas)).astype(int)

_prog_cache = {}


def _build_band_consts():
    """Host-side constant matrices for the matmuls (numpy float16)."""
    S = np.zeros((D, D), np.float32)
    idx = np.arange(D)
    S[idx, idx] = -60.0 * C / 12.0 + 2.0     # 2*p1 folded into the diagonal
    S[idx[:-1], idx[:-1] + 1] = 16.0 * C / 12.0
    S[idx[1:], idx[1:] - 1] = 16.0 * C / 12.0
    S[idx[:-2], idx[:-2] + 2] = -C / 12.0
    S[idx[2:], idx[2:] - 2] = -C / 12.0

    # BD00 [119,128]: band chunk0->chunk0 + aug selection + G-annex identity
    BD00 = np.zeros((KA, 128), np.float16)
    BD00[0:K, 0:K] = S[0:K, 0:K].T
    for s in range(48):
        BD00[K + s, COFF + s] = 1.0
    # BD10 [71,128]: band chunk1->chunk0 (cross) + aug selection
    BD10 = np.zeros((K, 128), np.float16)
    BD10[0:K, 0:K] = S[0:K, K:2 * K].T
    for i in range(NMEAS):
        g = _MX[i] - LO
        if g < K:
            BD00[g, 96 + i] = 1.0
        else:
            BD10[g - K, 96 + i] = 1.0
    # BD01 [71,71]: band chunk0->chunk1 (cross)
    BD01 = np.ascontiguousarray(S[K:2 * K, 0:K].T.astype(np.float16))
    # BD11 [119,71]: band chunk1->chunk1 + G-annex identity
    BD11 = np.zeros((KA, K), np.float16)
    BD11[0:K, 0:K] = S[K:2 * K, K:2 * K].T
    for s in range(48):
        BD11[K + s, s] = 1.0

    SH1 = (np.eye(K) * (16.0 * C / 12.0)).astype(np.float16)
    SH2 = (np.eye(K) * (-C / 12.0)).astype(np.float16)
    # SH1X0/SH1X1 [119,71]: sh1(+1) + G-annex identity rows. Riding the
    # shifts (block positions 3 and 8) instead of the band matmuls delays
    # the first annex consumer and releases the annex WAR mid-block, so the
    # G DMAs stop stalling the PE wait queue. The +1 rhs col offset is
    # pre-compensated in the host-side G column placement.
    SH1X0 = np.zeros((KA, K), np.float16)
    SH1X0[0:K, 0:K] = SH1
    for s in range(48):
        SH1X0[K + s, COFF + s] = 1.0         # G chunk0: out row 23+s
    SH1X1 = np.zeros((KA, K), np.float16)
    SH1X1[0:K, 0:K] = SH1
    for s in range(48):
        SH1X1[K + s, s] = 1.0                # G chunk1: out row s (=71+s)
    return {"BD00": BD00, "BD10": BD10, "BD01": BD01, "BD11": BD11,
            "SH1": SH1, "SH2": SH2}


def _build_program(nt=NT, reps=1):
    import concourse.bacc as bacc
    import concourse.tile as tile
    import concourse.mybir as mybir

    dt = mybir.dt
    nc = bacc.Bacc("TRN2", target_bir_lowering=False)

    G_d = nc.dram_tensor("G", (NT, BB, 2, 48, CW), dt.float16,
                         kind="ExternalInput")
    CSHAPES = [("BD00", (KA, 128)), ("BD10", (K, 128)),
               ("BD01", (K, K)), ("BD11", (KA, K)), ("SH1", (K, K)),
               ("SH2", (K, K))]
    CWID = sum(s[1][1] for s in CSHAPES)
    CP_d = nc.dram_tensor("CPACK", (KA, CWID), dt.float16,
                          kind="ExternalInput")
    MEAS_d = nc.dram_tensor("MEAS", (BB, NMEAS, NT // 2, 2 * SEG), dt.float16,
                            kind="ExternalOutput")

    with tile.TileContext(nc) as tc, ExitStack() as ctx:
        def sbuf(name, shape, dty):
            return ctx.enter_context(nc.sbuf_tensor(name, shape, dty))

        # state tiles: [chunk rows + G annex, chunk, cols]; T[f][s]
        T = [[sbuf(f"T{f}{s}", [KA, 2, TW], dt.float16) for s in range(2)]
             for f in range(BB)]
        cpack = sbuf("cpack", [KA, CWID], dt.float16)
        ct, _co = {}, 0
        for n, shp in CSHAPES:
            ct[n] = cpack[0:shp[0], _co:_co + shp[1]]
            _co += shp[1]
        # step-pair packing: contiguous [2, SEG] per (receiver, pair) gives
        # 584B DMA descriptor runs (>=512B avoids the 2x small-desc penalty)
        msb = [sbuf(f"msb{f}", [NMEAS, NT // 2, 2 * SEG], dt.float16)
               for f in range(BB)]

        # 8 one-bank PSUM tensors: PS[field][chunk][parity]
        PS = [[[ctx.enter_context(
                    nc.psum_tensor(f"PS{f}{kc}{p}", [128, 512], dt.float32))
                for p in range(2)] for kc in range(2)] for f in range(BB)]

        for f in range(BB):
            for s in range(2):
                (nc.vector if f == 0 else nc.gpsimd).memset(T[f][s][:], 0.0)
        nc.sync.dma_start(cpack[:], CP_d[:])

        def g_dma(q, f, s):
            """DMA G[q] for field f into tile slot s's annex (both chunks).
            Both ride the SP queue; the annex WAR releases mid-block (the
            +1 shifts are the readers) so neither issue waits long."""
            eng = nc.sync
            eng.dma_start(
                T[f][s][K:KA, 0:2, PAD + 2 + COFF: PAD + 2 + COFF + CW],
                G_d[q, f].rearrange("k p c -> p k c"))

        def rv(t, kc, off=0, annex=False):
            """Matmul-rhs view: [71 or 119, 146] run at col-tap off."""
            return t[0:(KA if annex else K), kc, PAD + off: PAD + off + SEG]

        cur, prev = 0, 1
        for rep in range(reps):
          if rep > 0:
            for f in range(BB):
                for s in range(2):
                    (nc.vector if f == 0 else nc.gpsimd).memset(T[f][s][:], 0.0)
          for j in range(nt):
              p = j % 2
              if j == 0:
                  for f in range(BB):
                      g_dma(0, f, cur)
                      if nt > 1:
                          g_dma(1, f, prev)

              for f in range(BB):
                  tc_, tp_ = T[f][cur], T[f][prev]
                  O0 = PS[f][0][p]
                  O1 = PS[f][1][p]
                  mm = nc.tensor.matmul
                  kw = dict(start=False, stop=False, skip_group_check=True)
                  # chunk0: band+G+aug opens the bank; cross (needs the other
                  # chunk's copyback of last step) closes it as late as ok.
                  mm(O0[0:128, 0:SEG], ct["BD00"], rv(tc_, 0, annex=True),
                     start=True, stop=False, skip_group_check=True)
                  mm(O0[0:K, 0:SEG], ct["SH1"], rv(tc_, 0, -1), **kw)
                  mm(O0[0:K, 0:SEG], ct["SH1"], rv(tc_, 0, 1), **kw)
                  mm(O0[0:K, 0:SEG], ct["SH2"], rv(tc_, 0, -2), **kw)
                  mm(O0[0:K, 0:SEG], ct["SH2"], rv(tc_, 0, 2), **kw)
                  mm(O1[0:K, 0:SEG], ct["BD01"], rv(tc_, 0),
                     start=True, stop=False, skip_group_check=True)
                  mm(O0[0:128, 0:SEG], ct["BD10"], rv(tc_, 1),
                     start=False, stop=True, skip_group_check=True)
                  mm(O1[0:K, 0:SEG], ct["SH1"], rv(tc_, 1, -1), **kw)
                  mm(O1[0:K, 0:SEG], ct["SH1"], rv(tc_, 1, 1), **kw)
                  mm(O1[0:K, 0:SEG], ct["SH2"], rv(tc_, 1, -2), **kw)
                  mm(O1[0:K, 0:SEG], ct["SH2"], rv(tc_, 1, 2), **kw)
                  mm(O1[0:K, 0:SEG], ct["BD11"], rv(tc_, 1, annex=True),
                     start=False, stop=True, skip_group_check=True)

                  for kc in range(2):
                      dv = tp_[0:K, kc, PAD + 2: PAD + 2 + D]
                      nc.vector.tensor_tensor(
                          out=dv, in0=PS[f][kc][p][0:K, 2:2 + D], in1=dv,
                          op=mybir.AluOpType.subtract)

                  if rep == reps - 1 and j > 0:
                      nc.scalar.copy(msb[f][:, (j - 1) // 2,
                                         ((j - 1) % 2) * SEG:((j - 1) % 2) * SEG + SEG],
                                     PS[f][0][p][96:96 + NMEAS, 0:SEG])
                  if j + 2 < nt:
                      g_dma(j + 2, f, cur)

              if rep == reps - 1 and j > 16 and j % 16 in (1, 5):
                  f = 0 if j % 16 == 1 else 1
                  ph = 1 if f == 0 else 5
                  b = (j - ph) // 16 - 1
                  nc.gpsimd.dma_start(
                      MEAS_d[f][:, 8 * b:8 * b + 8],
                      msb[f][:, 8 * b:8 * b + 8])

              cur, prev = prev, cur

        # post-loop: one more aug pair per field for p^(nt) -> slot nt-1
        for f in range(BB):
            O0 = PS[f][0][nt % 2]
            nc.tensor.matmul(O0[0:128, 0:SEG], ct["BD00"],
                             rv(T[f][cur], 0, annex=True),
                             start=True, stop=False, skip_group_check=True)
            nc.tensor.matmul(O0[0:128, 0:SEG], ct["BD10"],
                             rv(T[f][cur], 1),
                             start=False, stop=True, skip_group_check=True)
        for f in range(BB):
            nc.scalar.copy(msb[f][:, (nt - 1) // 2,
                               ((nt - 1) % 2) * SEG:((nt - 1) % 2) * SEG + SEG],
                           PS[f][0][nt % 2][96:96 + NMEAS, 0:SEG])
        for f in range(BB):
            ph = 1 if f == 0 else 5
            done = 8 * len([q for q in range(nt) if q > 16 and q % 16 == ph])
            nc.sync.dma_start(MEAS_d[f][:, done:nt // 2],
                              msb[f][:, done:nt // 2])

    nc.compile()
    return nc


def kernel(x, P0):
    x = np.asarray(x, dtype=np.float32)
    P0 = np.asarray(P0, dtype=np.float32)
    from concourse.bass_utils import run_bass_kernel_spmd

    if "prog" not in _prog_cache:
        _prog_cache["prog"] = _build_program()
    nc = _prog_cache["prog"]

    cb = _build_band_consts()
    order = ["BD00", "BD10", "BD01", "BD11", "SH1", "SH2"]
    wid = sum(cb[n].shape[1] for n in order)
    cp = np.zeros((KA, wid), np.float16)
    co = 0
    for n in order:
        a = cb[n]
        cp[0:a.shape[0], co:co + a.shape[1]] = a
        co += a.shape[1]
    consts = {"CPACK": cp}

    xx = bg / x[:, 0]
    rf = (1.0 - xx * xx).astype(np.float32)           # (B, 96, 96)
    P0c = P0[0, :, :, CLO:CHI, CLO:CHI]               # (NR, NT, 96, 96)
    d2 = np.zeros_like(P0c)
    d2[:, 2:] = P0c[:, 2:] - 2.0 * P0c[:, 1:-1] + P0c[:, :-2]

    in_maps = []
    for r in range(NRR):
        Gc = (rf[None, :, :, :] * d2[r][:, None, :, :]).astype(np.float16)
        G = np.zeros((NT, BB, 2, 48, CW), np.float16)
        G[:, :, 0] = Gc[:, :, 0:48, :]                # central rows 0..47
        G[:, :, 1] = Gc[:, :, 48:96, :]               # central rows 48..95
        m = dict(consts)
        m["G"] = G
        in_maps.append(m)

    trace = bool(int(os.environ.get("KERNEL_TRACE", "0")))
    res = run_bass_kernel_spmd(nc, in_maps, core_ids=list(range(NRR)),
                               trace=trace)
    _prog_cache["last_result"] = res

    ry = _MY - LO
    ii = np.arange(NMEAS)
    out = np.zeros((BB, NRR, NMEAS, NT), np.float32)
    for r in range(NRR):
        Ms = res.results[r]["MEAS"].reshape(BB, NMEAS, NT, SEG)
        for f in range(BB):
            out[f, r] = Ms[f][ii, :, 2 + ry].astype(np.float32)
    return out


# revision 30
# speedup vs baseline: 1.1951x; 1.0577x over previous
"""Trainium2 Bass kernel for nn_BornFoward: 200-step leapfrog wave recurrence.

Math (validated against the jax reference in a numpy model):
  - coef = (dt*BGF/dx)^2 is 0.2025 in the interior square [25:167)^2 of the
    192x192 grid and ~4.4e-13 in the outer absorbing ring; rf is EXACTLY zero
    outside the central 96x96 window (pad region has X==1 -> 1-X^2==0).
  - Therefore the recurrence restricted to the 142x142 interior with zero
    Dirichlet boundary and constant coef reproduces the reference to ~1e-9.
  - p_new = 2*p1 - p0 + C*lap4(p1) + rf*d2(P0),  meas = p_new at 32 pixels.
  - fp16 state/weights/source keep the end-to-end error at ~9e-3 of the
    output scale (validated in numpy), under the 2e-2 gate.

Sharding: 16 independent recurrences (B=2 x NR=8) -> channel r per core.

The two batch fields are INDEPENDENT recurrences: they are interleaved as
two separate matmul streams (fp16 runs 1 cycle/row at any free size, so the
N>=256 float32r constraint that forced batching them in one 292-wide stream
is gone). Each field's PSUM->DVE-copyback->next-matmul latency (~900ns)
hides under the other field's 12-matmul block.

Per field, per step (parity p = j%2, PSUM bank tensor PS[f][chunk][p]):
  - state tiles [119, 2(chunk), 154] fp16: partitions 0..70 hold rows of the
    chunk ([4 pad | 2 guard | 142 | 2 guard | 4 pad] cols); partitions
    71..118 are a G ANNEX holding the 48 central rows of rf*d2 for this
    step, zero outside the central cols. The band lhsT is extended with 48
    identity rows, so the source term rides the band matmul's contraction
    dim -- G delivery costs zero extra PE/DVE/Act time; it is DMA'd from
    DRAM straight into the annex (1 DMA per field per step, 2-step lookahead
    into the tile that will be cur again).
  - 12 matmuls: band+G+aug (start=True zeroes the bank), 4 shifted-identity
    y-stencil taps, cross band from the other chunk (stop=True), x2 chunks.
    The chunk-0 band matmuls carry 32 augmented selection columns that copy
    the receiver rows of the current state into PSUM rows 96..127.
  - p_new = PSUM - p0: DVE copyback per chunk (rotates state tiles).
  - the Activation engine copies the [32, 146] selection rows into a big
    on-chip measurement buffer (fp16, lossless: they are copies of fp16
    state values); ONE DMA at the end ships all measurements; the host
    picks each receiver's column. No per-step measurement DMA/mask-reduce.
"""
import sys
import os
import numpy as np
from contextlib import ExitStack

sys.path.insert(0, "/opt/trn_rl_repo")

# ---- problem constants (hardcoded; kernel.py must be self-contained) ----
NX = 192
NT = 200
dtime = 0.3
nm, sR = 32, 70
bg = 1.5
LO, HI = 25, 167            # interior rows/cols [LO, HI) -> D = 142
D = HI - LO
CLO, CHI = 48, 144          # central rf-support window (96 wide)
CW = CHI - CLO
COFF = CLO - LO             # 23: central window offset inside domain
C = (dtime * bg / 1.0) ** 2  # 0.2025
K = 71                      # row-chunk size (2 chunks of 71 = 142)
KA = K + 48                 # 119: chunk rows + G annex rows
SEG = 2 + D + 2             # 146: per-chunk col run with 2-col guards
NRR = 8
BB = 2
NMEAS = nm
PAD = 4                     # left/right pad so shift views stay in-bounds
TW = PAD + SEG + PAD        # 154: state tile cols per chunk

_thetas = 2 * np.pi * np.arange(nm) / nm
_MX = (NX / 2 + sR * np.cos(_thetas)).astype(int)
_MY = (NX / 2 + sR * np.sin(_thetas)).astype(int)

_prog_cache = {}


def _build_band_consts():
    """Host-side constant matrices for the matmuls (numpy float16)."""
    S = np.zeros((D, D), np.float32)
    idx = np.arange(D)
    S[idx, idx] = -60.0 * C / 12.0 + 2.0     # 2*p1 folded into the diagonal
    S[idx[:-1], idx[:-1] + 1] = 16.0 * C / 12.0
    S[idx[1:], idx[1:] - 1] = 16.0 * C / 12.0
    S[idx[:-2], idx[:-2] + 2] = -C / 12.0
    S[idx[2:], idx[2:] - 2] = -C / 12.0

    # BD00 [71,128]: band chunk0->chunk0 + aug selection
    BD00 = np.zeros((K, 128), np.float16)
    BD00[0:K, 0:K] = S[0:K, 0:K].T
    # BD10 [119,128]: band chunk1->chunk0 (cross) + aug + G-annex identity.
    # The G annexes ride the two CROSS matmuls (block positions 6-7): the
    # consumer sits mid-block and the annex WAR releases mid-block, so the
    # 2-step-lookahead G DMA neither stalls the PE wait queue nor
    # head-of-line-blocks the SP queue.
    BD10 = np.zeros((KA, 128), np.float16)
    BD10[0:K, 0:K] = S[0:K, K:2 * K].T
    for s in range(48):
        BD10[K + s, COFF + s] = 1.0          # G chunk0: out row 23+s
    for i in range(NMEAS):
        g = _MX[i] - LO
        if g < K:
            BD00[g, 96 + i] = 1.0
        else:
            BD10[g - K, 96 + i] = 1.0
    # BD01 [119,71]: band chunk0->chunk1 (cross) + G-annex identity
    BD01 = np.zeros((KA, K), np.float16)
    BD01[0:K, 0:K] = S[K:2 * K, 0:K].T
    for s in range(48):
        BD01[K + s, s] = 1.0                 # G chunk1: out row s (=71+s)
    # BD11 [71,71]: band chunk1->chunk1
    BD11 = np.ascontiguousarray(S[K:2 * K, K:2 * K].T.astype(np.float16))

    SH1 = (np.eye(K) * (16.0 * C / 12.0)).astype(np.float16)
    SH2 = (np.eye(K) * (-C / 12.0)).astype(np.float16)
    # SH1X0/SH1X1 [119,71]: sh1(+1) + G-annex identity rows. Riding the
    # shifts (block positions 3 and 8) instead of the band matmuls delays
    # the first annex consumer and releases the annex WAR mid-block, so the
    # G DMAs stop stalling the PE wait queue. The +1 rhs col offset is
    # pre-compensated in the host-side G column placement.
    SH1X0 = np.zeros((KA, K), np.float16)
    SH1X0[0:K, 0:K] = SH1
    for s in range(48):
        SH1X0[K + s, COFF + s] = 1.0         # G chunk0: out row 23+s
    SH1X1 = np.zeros((KA, K), np.float16)
    SH1X1[0:K, 0:K] = SH1
    for s in range(48):
        SH1X1[K + s, s] = 1.0                # G chunk1: out row s (=71+s)
    return {"BD00": BD00, "BD10": BD10, "BD01": BD01, "BD11": BD11,
            "SH1": SH1, "SH2": SH2}


def _build_program(nt=NT, reps=1):
    import concourse.bacc as bacc
    import concourse.tile as tile
    import concourse.mybir as mybir

    dt = mybir.dt
    nc = bacc.Bacc("TRN2", target_bir_lowering=False)

    G_d = nc.dram_tensor("G", (NT, BB, 2, 48, CW), dt.float16,
                         kind="ExternalInput")
    CSHAPES = [("BD00", (K, 128)), ("BD10", (KA, 128)),
               ("BD01", (KA, K)), ("BD11", (K, K)), ("SH1", (K, K)),
               ("SH2", (K, K))]
    CWID = sum(s[1][1] for s in CSHAPES)
    CP_d = nc.dram_tensor("CPACK", (KA, CWID), dt.float16,
                          kind="ExternalInput")
    MEAS_d = nc.dram_tensor("MEAS", (BB, NMEAS, NT // 2, 2 * SEG), dt.float16,
                            kind="ExternalOutput")

    with tile.TileContext(nc) as tc, ExitStack() as ctx:
        def sbuf(name, shape, dty):
            return ctx.enter_context(nc.sbuf_tensor(name, shape, dty))

        # state tiles: [chunk rows + G annex, chunk, cols]; T[f][s]
        T = [[sbuf(f"T{f}{s}", [KA, 2, TW], dt.float16) for s in range(2)]
             for f in range(BB)]
        cpack = sbuf("cpack", [KA, CWID], dt.float16)
        ct, _co = {}, 0
        for n, shp in CSHAPES:
            ct[n] = cpack[0:shp[0], _co:_co + shp[1]]
            _co += shp[1]
        # step-pair packing: contiguous [2, SEG] per (receiver, pair) gives
        # 584B DMA descriptor runs (>=512B avoids the 2x small-desc penalty)
        msb = [sbuf(f"msb{f}", [NMEAS, NT // 2, 2 * SEG], dt.float16)
               for f in range(BB)]

        # 8 one-bank PSUM tensors: PS[field][chunk][parity]
        PS = [[[ctx.enter_context(
                    nc.psum_tensor(f"PS{f}{kc}{p}", [128, 512], dt.float32))
                for p in range(2)] for kc in range(2)] for f in range(BB)]

        for f in range(BB):
            for s in range(2):
                (nc.vector if f == 0 else nc.gpsimd).memset(T[f][s][:], 0.0)
        nc.sync.dma_start(cpack[:], CP_d[:])

        def g_dma(q, f, s):
            """DMA G[q] for field f into tile slot s's annex (both chunks).
            Both ride the SP queue; the annex WAR releases mid-block (the
            +1 shifts are the readers) so neither issue waits long."""
            eng = nc.sync
            eng.dma_start(
                T[f][s][K:KA, 0:2, PAD + 2 + COFF: PAD + 2 + COFF + CW],
                G_d[q, f].rearrange("k p c -> p k c"))

        def rv(t, kc, off=0, annex=False):
            """Matmul-rhs view: [71 or 119, 146] run at col-tap off."""
            return t[0:(KA if annex else K), kc, PAD + off: PAD + off + SEG]

        cur, prev = 0, 1
        for rep in range(reps):
          if rep > 0:
            for f in range(BB):
                for s in range(2):
                    (nc.vector if f == 0 else nc.gpsimd).memset(T[f][s][:], 0.0)
          for j in range(nt):
              p = j % 2
              if j == 0:
                  for f in range(BB):
                      g_dma(0, f, cur)
                      if nt > 1:
                          g_dma(1, f, prev)

              for f in range(BB):
                  tc_, tp_ = T[f][cur], T[f][prev]
                  O0 = PS[f][0][p]
                  O1 = PS[f][1][p]
                  mm = nc.tensor.matmul
                  kw = dict(start=False, stop=False, skip_group_check=True)
                  # chunk0: band+G+aug opens the bank; cross (needs the other
                  # chunk's copyback of last step) closes it as late as ok.
                  mm(O0[0:128, 0:SEG], ct["BD00"], rv(tc_, 0),
                     start=True, stop=False, skip_group_check=True)
                  mm(O0[0:K, 0:SEG], ct["SH1"], rv(tc_, 0, -1), **kw)
                  mm(O0[0:K, 0:SEG], ct["SH1"], rv(tc_, 0, 1), **kw)
                  mm(O0[0:K, 0:SEG], ct["SH2"], rv(tc_, 0, -2), **kw)
                  mm(O0[0:K, 0:SEG], ct["SH2"], rv(tc_, 0, 2), **kw)
                  mm(O1[0:K, 0:SEG], ct["BD01"], rv(tc_, 0, annex=True),
                     start=True, stop=False, skip_group_check=True)
                  mm(O0[0:128, 0:SEG], ct["BD10"], rv(tc_, 1, annex=True),
                     start=False, stop=True, skip_group_check=True)
                  mm(O1[0:K, 0:SEG], ct["SH1"], rv(tc_, 1, -1), **kw)
                  mm(O1[0:K, 0:SEG], ct["SH1"], rv(tc_, 1, 1), **kw)
                  mm(O1[0:K, 0:SEG], ct["SH2"], rv(tc_, 1, -2), **kw)
                  mm(O1[0:K, 0:SEG], ct["SH2"], rv(tc_, 1, 2), **kw)
                  mm(O1[0:K, 0:SEG], ct["BD11"], rv(tc_, 1),
                     start=False, stop=True, skip_group_check=True)

                  for kc in range(2):
                      dv = tp_[0:K, kc, PAD + 2: PAD + 2 + D]
                      nc.vector.tensor_tensor(
                          out=dv, in0=PS[f][kc][p][0:K, 2:2 + D], in1=dv,
                          op=mybir.AluOpType.subtract)

                  if rep == reps - 1 and j > 0:
                      nc.scalar.copy(msb[f][:, (j - 1) // 2,
                                         ((j - 1) % 2) * SEG:((j - 1) % 2) * SEG + SEG],
                                     PS[f][0][p][96:96 + NMEAS, 0:SEG])
                  if j + 2 < nt:
                      g_dma(j + 2, f, cur)

              if rep == reps - 1 and j > 16 and j % 16 in (1, 5):
                  f = 0 if j % 16 == 1 else 1
                  ph = 1 if f == 0 else 5
                  b = (j - ph) // 16 - 1
                  nc.gpsimd.dma_start(
                      MEAS_d[f][:, 8 * b:8 * b + 8],
                      msb[f][:, 8 * b:8 * b + 8])

              cur, prev = prev, cur

        # post-loop: one more aug pair per field for p^(nt) -> slot nt-1
        for f in range(BB):
            O0 = PS[f][0][nt % 2]
            nc.tensor.matmul(O0[0:128, 0:SEG], ct["BD00"],
                             rv(T[f][cur], 0),
                             start=True, stop=False, skip_group_check=True)
            nc.tensor.matmul(O0[0:128, 0:SEG], ct["BD10"],
                             rv(T[f][cur], 1, annex=True),
                             start=False, stop=True, skip_group_check=True)
        for f in range(BB):
            nc.scalar.copy(msb[f][:, (nt - 1) // 2,
                               ((nt - 1) % 2) * SEG:((nt - 1) % 2) * SEG + SEG],
                           PS[f][0][nt % 2][96:96 + NMEAS, 0:SEG])
        for f in range(BB):
            ph = 1 if f == 0 else 5
            done = 8 * len([q for q in range(nt) if q > 16 and q % 16 == ph])
            nc.sync.dma_start(MEAS_d[f][:, done:nt // 2],
                              msb[f][:, done:nt // 2])

    nc.compile()
    return nc


def kernel(x, P0):
    x = np.asarray(x, dtype=np.float32)
    P0 = np.asarray(P0, dtype=np.float32)
    from concourse.bass_utils import run_bass_kernel_spmd

    if "prog" not in _prog_cache:
        _prog_cache["prog"] = _build_program()
    nc = _prog_cache["prog"]

    cb = _build_band_consts()
    order = ["BD00", "BD10", "BD01", "BD11", "SH1", "SH2"]
    wid = sum(cb[n].shape[1] for n in order)
    cp = np.zeros((KA, wid), np.float16)
    co = 0
    for n in order:
        a = cb[n]
        cp[0:a.shape[0], co:co + a.shape[1]] = a
        co += a.shape[1]
    consts = {"CPACK": cp}

    xx = bg / x[:, 0]
    rf = (1.0 - xx * xx).astype(np.float32)           # (B, 96, 96)
    P0c = P0[0, :, :, CLO:CHI, CLO:CHI]               # (NR, NT, 96, 96)
    d2 = np.zeros_like(P0c)
    d2[:, 2:] = P0c[:, 2:] - 2.0 * P0c[:, 1:-1] + P0c[:, :-2]

    in_maps = []
    for r in range(NRR):
        Gc = (rf[None, :, :, :] * d2[r][:, None, :, :]).astype(np.float16)
        G = np.zeros((NT, BB, 2, 48, CW), np.float16)
        G[:, :, 0] = Gc[:, :, 48:96, :]   # tile chunk-0 annex -> bd01 -> rows 71+
        G[:, :, 1] = Gc[:, :, 0:48, :]    # tile chunk-1 annex -> bd10 -> rows 23+
        m = dict(consts)
        m["G"] = G
        in_maps.append(m)

    trace = bool(int(os.environ.get("KERNEL_TRACE", "0")))
    res = run_bass_kernel_spmd(nc, in_maps, core_ids=list(range(NRR)),
                               trace=trace)
    _prog_cache["last_result"] = res

    ry = _MY - LO
    ii = np.arange(NMEAS)
    out = np.zeros((BB, NRR, NMEAS, NT), np.float32)
    for r in range(NRR):
        Ms = res.results[r]["MEAS"].reshape(BB, NMEAS, NT, SEG)
        for f in range(BB):
            out[f, r] = Ms[f][ii, :, 2 + ry].astype(np.float32)
    return out


# revision 31
# speedup vs baseline: 1.2338x; 1.0324x over previous
"""Trainium2 Bass kernel for nn_BornFoward: 200-step leapfrog wave recurrence.

Math (validated against the jax reference in a numpy model):
  - coef = (dt*BGF/dx)^2 is 0.2025 in the interior square [25:167)^2 of the
    192x192 grid and ~4.4e-13 in the outer absorbing ring; rf is EXACTLY zero
    outside the central 96x96 window (pad region has X==1 -> 1-X^2==0).
  - Therefore the recurrence restricted to the 142x142 interior with zero
    Dirichlet boundary and constant coef reproduces the reference to ~1e-9.
  - p_new = 2*p1 - p0 + C*lap4(p1) + rf*d2(P0),  meas = p_new at 32 pixels.
  - fp16 state/weights/source keep the end-to-end error at ~9e-3 of the
    output scale (validated in numpy), under the 2e-2 gate.

Sharding: 16 independent recurrences (B=2 x NR=8) -> channel r per core.

The two batch fields are INDEPENDENT recurrences: they are interleaved as
two separate matmul streams (fp16 runs 1 cycle/row at any free size, so the
N>=256 float32r constraint that forced batching them in one 292-wide stream
is gone). Each field's PSUM->DVE-copyback->next-matmul latency (~900ns)
hides under the other field's 12-matmul block.

Per field, per step (parity p = j%2, PSUM bank tensor PS[f][chunk][p]):
  - state tiles [119, 2(chunk), 154] fp16: partitions 0..70 hold rows of the
    chunk ([4 pad | 2 guard | 142 | 2 guard | 4 pad] cols); partitions
    71..118 are a G ANNEX holding the 48 central rows of rf*d2 for this
    step, zero outside the central cols. The band lhsT is extended with 48
    identity rows, so the source term rides the band matmul's contraction
    dim -- G delivery costs zero extra PE/DVE/Act time; it is DMA'd from
    DRAM straight into the annex (1 DMA per field per step, 2-step lookahead
    into the tile that will be cur again).
  - 12 matmuls: band+G+aug (start=True zeroes the bank), 4 shifted-identity
    y-stencil taps, cross band from the other chunk (stop=True), x2 chunks.
    The chunk-0 band matmuls carry 32 augmented selection columns that copy
    the receiver rows of the current state into PSUM rows 96..127.
  - p_new = PSUM - p0: DVE copyback per chunk (rotates state tiles).
  - the Activation engine copies the [32, 146] selection rows into a big
    on-chip measurement buffer (fp16, lossless: they are copies of fp16
    state values); ONE DMA at the end ships all measurements; the host
    picks each receiver's column. No per-step measurement DMA/mask-reduce.
"""
import sys
import os
import numpy as np
from contextlib import ExitStack

sys.path.insert(0, "/opt/trn_rl_repo")

# ---- problem constants (hardcoded; kernel.py must be self-contained) ----
NX = 192
NT = 200
dtime = 0.3
nm, sR = 32, 70
bg = 1.5
LO, HI = 25, 167            # interior rows/cols [LO, HI) -> D = 142
D = HI - LO
CLO, CHI = 48, 144          # central rf-support window (96 wide)
CW = CHI - CLO
COFF = CLO - LO             # 23: central window offset inside domain
C = (dtime * bg / 1.0) ** 2  # 0.2025
K = 71                      # row-chunk size (2 chunks of 71 = 142)
KA = K + 48                 # 119: chunk rows + G annex rows
SEG = 2 + D + 2             # 146: per-chunk col run with 2-col guards
NRR = 8
BB = 2
NMEAS = nm
PAD = 4                     # left/right pad so shift views stay in-bounds
TW = PAD + SEG + PAD        # 154: state tile cols per chunk

_thetas = 2 * np.pi * np.arange(nm) / nm
_MX = (NX / 2 + sR * np.cos(_thetas)).astype(int)
_MY = (NX / 2 + sR * np.sin(_thetas)).astype(int)

_prog_cache = {}


def _build_band_consts():
    """Host-side constant matrices for the matmuls (numpy float16)."""
    S = np.zeros((D, D), np.float32)
    idx = np.arange(D)
    S[idx, idx] = -60.0 * C / 12.0 + 2.0     # 2*p1 folded into the diagonal
    S[idx[:-1], idx[:-1] + 1] = 16.0 * C / 12.0
    S[idx[1:], idx[1:] - 1] = 16.0 * C / 12.0
    S[idx[:-2], idx[:-2] + 2] = -C / 12.0
    S[idx[2:], idx[2:] - 2] = -C / 12.0

    # BD00 [71,128]: band chunk0->chunk0 + aug selection
    BD00 = np.zeros((K, 128), np.float16)
    BD00[0:K, 0:K] = S[0:K, 0:K].T
    # BD10 [119,128]: band chunk1->chunk0 (cross) + aug + G-annex identity.
    # The G annexes ride the two CROSS matmuls (block positions 6-7): the
    # consumer sits mid-block and the annex WAR releases mid-block, so the
    # 2-step-lookahead G DMA neither stalls the PE wait queue nor
    # head-of-line-blocks the SP queue.
    BD10 = np.zeros((KA, 128), np.float16)
    BD10[0:K, 0:K] = S[0:K, K:2 * K].T
    for s in range(48):
        BD10[K + s, COFF + s] = 1.0          # G chunk0: out row 23+s
    for i in range(NMEAS):
        g = _MX[i] - LO
        if g < K:
            BD00[g, 96 + i] = 1.0
        else:
            BD10[g - K, 96 + i] = 1.0
    # BD01 [119,71]: band chunk0->chunk1 (cross) + G-annex identity
    BD01 = np.zeros((KA, K), np.float16)
    BD01[0:K, 0:K] = S[K:2 * K, 0:K].T
    for s in range(48):
        BD01[K + s, s] = 1.0                 # G chunk1: out row s (=71+s)
    # BD11 [71,71]: band chunk1->chunk1
    BD11 = np.ascontiguousarray(S[K:2 * K, K:2 * K].T.astype(np.float16))

    SH1 = (np.eye(K) * (16.0 * C / 12.0)).astype(np.float16)
    SH2 = (np.eye(K) * (-C / 12.0)).astype(np.float16)
    # SH1X0/SH1X1 [119,71]: sh1(+1) + G-annex identity rows. Riding the
    # shifts (block positions 3 and 8) instead of the band matmuls delays
    # the first annex consumer and releases the annex WAR mid-block, so the
    # G DMAs stop stalling the PE wait queue. The +1 rhs col offset is
    # pre-compensated in the host-side G column placement.
    SH1X0 = np.zeros((KA, K), np.float16)
    SH1X0[0:K, 0:K] = SH1
    for s in range(48):
        SH1X0[K + s, COFF + s] = 1.0         # G chunk0: out row 23+s
    SH1X1 = np.zeros((KA, K), np.float16)
    SH1X1[0:K, 0:K] = SH1
    for s in range(48):
        SH1X1[K + s, s] = 1.0                # G chunk1: out row s (=71+s)
    return {"BD00": BD00, "BD10": BD10, "BD01": BD01, "BD11": BD11,
            "SH1": SH1, "SH2": SH2}


def _build_program(nt=NT, reps=1):
    import concourse.bacc as bacc
    import concourse.tile as tile
    import concourse.mybir as mybir

    dt = mybir.dt
    nc = bacc.Bacc("TRN2", target_bir_lowering=False)

    G_d = nc.dram_tensor("G", (NT, BB, 2, 48, CW), dt.float16,
                         kind="ExternalInput")
    CSHAPES = [("BD00", (K, 128)), ("BD10", (KA, 128)),
               ("BD01", (KA, K)), ("BD11", (K, K)), ("SH1", (K, K)),
               ("SH2", (K, K))]
    CWID = sum(s[1][1] for s in CSHAPES)
    CP_d = nc.dram_tensor("CPACK", (KA, CWID), dt.float16,
                          kind="ExternalInput")
    MEAS_d = nc.dram_tensor("MEAS", (BB, NMEAS, NT // 2, 2 * SEG), dt.float16,
                            kind="ExternalOutput")

    with tile.TileContext(nc) as tc, ExitStack() as ctx:
        def sbuf(name, shape, dty):
            return ctx.enter_context(nc.sbuf_tensor(name, shape, dty))

        # state tiles: [chunk rows + G annex, chunk, cols]; T[f][s]
        T = [[sbuf(f"T{f}{s}", [KA, 2, TW], dt.float16) for s in range(2)]
             for f in range(BB)]
        cpack = sbuf("cpack", [KA, CWID], dt.float16)
        ct, _co = {}, 0
        for n, shp in CSHAPES:
            ct[n] = cpack[0:shp[0], _co:_co + shp[1]]
            _co += shp[1]
        # step-pair packing: contiguous [2, SEG] per (receiver, pair) gives
        # 584B DMA descriptor runs (>=512B avoids the 2x small-desc penalty)
        msb = [sbuf(f"msb{f}", [NMEAS, NT // 2, 2 * SEG], dt.float16)
               for f in range(BB)]

        # 8 one-bank PSUM tensors: PS[field][chunk][parity]
        PS = [[[ctx.enter_context(
                    nc.psum_tensor(f"PS{f}{kc}{p}", [128, 512], dt.float32))
                for p in range(2)] for kc in range(2)] for f in range(BB)]

        for f in range(BB):
            for s in range(2):
                (nc.vector if f == 0 else nc.gpsimd).memset(T[f][s][:], 0.0)
        nc.sync.dma_start(cpack[:], CP_d[:])

        def g_dma(q, f, s):
            """DMA G[q] for field f into tile slot s's annex (both chunks).
            Both ride the SP queue; the annex WAR releases mid-block (the
            +1 shifts are the readers) so neither issue waits long."""
            eng = nc.sync
            eng.dma_start(
                T[f][s][K:KA, 0:2, PAD + 2 + COFF: PAD + 2 + COFF + CW],
                G_d[q, f].rearrange("k p c -> p k c"))

        def rv(t, kc, off=0, annex=False):
            """Matmul-rhs view: [71 or 119, 142] run at col-tap off.
            N=142 covers exactly the data cols (out region [2:144] of the
            bank); the 2-col guards are still READ by the off<0 / off>0
            views but no longer streamed as output columns."""
            return t[0:(KA if annex else K), kc,
                     PAD + 2 + off: PAD + 2 + off + D]

        cur, prev = 0, 1
        for rep in range(reps):
          if rep > 0:
            for f in range(BB):
                for s in range(2):
                    (nc.vector if f == 0 else nc.gpsimd).memset(T[f][s][:], 0.0)
          for j in range(nt):
              p = j % 2
              if j == 0:
                  for f in range(BB):
                      g_dma(0, f, cur)
                      if nt > 1:
                          g_dma(1, f, prev)

              for f in range(BB):
                  tc_, tp_ = T[f][cur], T[f][prev]
                  O0 = PS[f][0][p]
                  O1 = PS[f][1][p]
                  mm = nc.tensor.matmul
                  kw = dict(start=False, stop=False, skip_group_check=True)
                  # chunk0: band+G+aug opens the bank; cross (needs the other
                  # chunk's copyback of last step) closes it as late as ok.
                  mm(O0[0:128, 2:2 + D], ct["BD00"], rv(tc_, 0),
                     start=True, stop=False, skip_group_check=True)
                  mm(O0[0:K, 2:2 + D], ct["SH1"], rv(tc_, 0, -1), **kw)
                  mm(O0[0:K, 2:2 + D], ct["SH1"], rv(tc_, 0, 1), **kw)
                  mm(O0[0:K, 2:2 + D], ct["SH2"], rv(tc_, 0, -2), **kw)
                  mm(O0[0:K, 2:2 + D], ct["SH2"], rv(tc_, 0, 2), **kw)
                  mm(O1[0:K, 2:2 + D], ct["BD01"], rv(tc_, 0, annex=True),
                     start=True, stop=False, skip_group_check=True)
                  mm(O0[0:128, 2:2 + D], ct["BD10"], rv(tc_, 1, annex=True),
                     start=False, stop=True, skip_group_check=True)
                  mm(O1[0:K, 2:2 + D], ct["SH1"], rv(tc_, 1, -1), **kw)
                  mm(O1[0:K, 2:2 + D], ct["SH1"], rv(tc_, 1, 1), **kw)
                  mm(O1[0:K, 2:2 + D], ct["SH2"], rv(tc_, 1, -2), **kw)
                  mm(O1[0:K, 2:2 + D], ct["SH2"], rv(tc_, 1, 2), **kw)
                  mm(O1[0:K, 2:2 + D], ct["BD11"], rv(tc_, 1),
                     start=False, stop=True, skip_group_check=True)

                  for kc in range(2):
                      dv = tp_[0:K, kc, PAD + 2: PAD + 2 + D]
                      nc.vector.tensor_tensor(
                          out=dv, in0=PS[f][kc][p][0:K, 2:2 + D], in1=dv,
                          op=mybir.AluOpType.subtract)

                  if rep == reps - 1 and j > 0:
                      nc.scalar.copy(msb[f][:, (j - 1) // 2,
                                         ((j - 1) % 2) * SEG:((j - 1) % 2) * SEG + SEG],
                                     PS[f][0][p][96:96 + NMEAS, 0:SEG])
                  if j + 2 < nt:
                      g_dma(j + 2, f, cur)

              if rep == reps - 1 and j > 16 and j % 16 in (1, 5):
                  f = 0 if j % 16 == 1 else 1
                  ph = 1 if f == 0 else 5
                  b = (j - ph) // 16 - 1
                  nc.gpsimd.dma_start(
                      MEAS_d[f][:, 8 * b:8 * b + 8],
                      msb[f][:, 8 * b:8 * b + 8])

              cur, prev = prev, cur

        # post-loop: one more aug pair per field for p^(nt) -> slot nt-1
        for f in range(BB):
            O0 = PS[f][0][nt % 2]
            nc.tensor.matmul(O0[0:128, 2:2 + D], ct["BD00"],
                             rv(T[f][cur], 0),
                             start=True, stop=False, skip_group_check=True)
            nc.tensor.matmul(O0[0:128, 2:2 + D], ct["BD10"],
                             rv(T[f][cur], 1, annex=True),
                             start=False, stop=True, skip_group_check=True)
        for f in range(BB):
            nc.scalar.copy(msb[f][:, (nt - 1) // 2,
                               ((nt - 1) % 2) * SEG:((nt - 1) % 2) * SEG + SEG],
                           PS[f][0][nt % 2][96:96 + NMEAS, 0:SEG])
        for f in range(BB):
            ph = 1 if f == 0 else 5
            done = 8 * len([q for q in range(nt) if q > 16 and q % 16 == ph])
            nc.sync.dma_start(MEAS_d[f][:, done:nt // 2],
                              msb[f][:, done:nt // 2])

    nc.compile()
    return nc


def kernel(x, P0):
    x = np.asarray(x, dtype=np.float32)
    P0 = np.asarray(P0, dtype=np.float32)
    from concourse.bass_utils import run_bass_kernel_spmd

    if "prog" not in _prog_cache:
        _prog_cache["prog"] = _build_program()
    nc = _prog_cache["prog"]

    cb = _build_band_consts()
    order = ["BD00", "BD10", "BD01", "BD11", "SH1", "SH2"]
    wid = sum(cb[n].shape[1] for n in order)
    cp = np.zeros((KA, wid), np.float16)
    co = 0
    for n in order:
        a = cb[n]
        cp[0:a.shape[0], co:co + a.shape[1]] = a
        co += a.shape[1]
    consts = {"CPACK": cp}

    xx = bg / x[:, 0]
    rf = (1.0 - xx * xx).astype(np.float32)           # (B, 96, 96)
    P0c = P0[0, :, :, CLO:CHI, CLO:CHI]               # (NR, NT, 96, 96)
    d2 = np.zeros_like(P0c)
    d2[:, 2:] = P0c[:, 2:] - 2.0 * P0c[:, 1:-1] + P0c[:, :-2]

    in_maps = []
    for r in range(NRR):
        Gc = (rf[None, :, :, :] * d2[r][:, None, :, :]).astype(np.float16)
        G = np.zeros((NT, BB, 2, 48, CW), np.float16)
        G[:, :, 0] = Gc[:, :, 48:96, :]   # tile chunk-0 annex -> bd01 -> rows 71+
        G[:, :, 1] = Gc[:, :, 0:48, :]    # tile chunk-1 annex -> bd10 -> rows 23+
        m = dict(consts)
        m["G"] = G
        in_maps.append(m)

    trace = bool(int(os.environ.get("KERNEL_TRACE", "0")))
    res = run_bass_kernel_spmd(nc, in_maps, core_ids=list(range(NRR)),
                               trace=trace)
    _prog_cache["last_result"] = res

    ry = _MY - LO
    ii = np.arange(NMEAS)
    out = np.zeros((BB, NRR, NMEAS, NT), np.float32)
    for r in range(NRR):
        Ms = res.results[r]["MEAS"].reshape(BB, NMEAS, NT, SEG)
        for f in range(BB):
            out[f, r] = Ms[f][ii, :, 2 + ry].astype(np.float32)
    return out
